# revision 24
# baseline (speedup 1.0000x reference)
"""HBMP (3-branch LSTM + BiLSTM + global max pool) Trainium2 kernel.

Model (B=64, T=512, E=300, H=512, NB=3 branches):
  per branch: h1 = LSTM(x); hf = LSTM(h1); hb = rev(LSTM(rev(h1)))
  emb = maxpool_T(concat([hf, hb], -1));  out = concat over branches [B, 3*2H]

Mapping onto 6 NeuronCores (task-parallel; batch stays whole because the
recurrent matmul cost is weight-streaming-bound, independent of batch):
  core c in 0..5 handles (branch = c%3, direction = fwd if c<3 else bwd):
    scan1: uni LSTM over x with the x-projection (x@Wx_u+b, bf16) fused
      into the per-step schedule via an SBUF ring (no DRAM xz roundtrip);
      h1^T is scattered to DRAM hT with per-core indices that REVERSE time
      for bwd cores, so scan2 always reads hT sequentially.
    scan2: dir LSTM; the input projection (h1@Wx_d+b) is fused here,
      re-reading hT (sequential) into the same SBUF ring scheme. Running
      max over h gives rmax [64, 512] directly (max is order-independent,
      so bwd cores never un-reverse).
Host gathers the 6 rmax outputs into [64, 3072].

Scan step (gate column order host-permuted to [f i g o]):
  z (PSUM, [64,4H]) = xz_t injected via identity matmul from the SBUF ring
  (id2 trick: lhsT = rows 0:64 or 64:128 of eye(128) selects the even/odd
  timestep packed on ring partitions), + h_{t-1} @ Wh as 4 K-tile matmuls
  per 512-wide gate bank, n-outer so early gate banks finish first;
  ScalarE applies sigmoid/tanh straight out of PSUM bank-by-bank while the
  PE finishes later banks; c/h updates on VectorE; h re-transposed on PE,
  PSUM->SBUF copy of hT on ScalarE (keeps DVE off the serial chain);
  running max on GpSimd. The fused projection matmuls give the PE filler
  work during the elementwise chain, keeping the HAM clock warm.

The compiled program is executed through a jit-cached shard_map wrapper
(trace/lower/compile once, reuse across calls); inputs can be pinned on
device for repeat timing runs.
"""
import sys

sys.path.insert(0, "/opt/trn_rl_repo")

import numpy as np

B, T, E, H = 64, 512, 300, 512
FOUR_H = 4 * H
NB = 3
N_CORES = 6

_CACHE = {}


def _build_program(rep=1):
    import concourse.bass as bass
    import concourse.tile as tile
    from concourse import bacc, mybir

    F32 = mybir.dt.float32
    F32R = mybir.dt.float32r
    BF16 = mybir.dt.bfloat16
    I32 = mybir.dt.int32
    Sig = mybir.ActivationFunctionType.Sigmoid
    Tanh = mybir.ActivationFunctionType.Tanh
    Copy = mybir.ActivationFunctionType.Copy

    nc = bacc.Bacc("TRN2", target_bir_lowering=False, debug=False,
                   enable_asserts=False, num_devices=N_CORES)

    d = {}
    d["xTu"] = nc.dram_tensor("xTu", [T, 128, 3, 64], BF16, kind="ExternalInput").ap()
    d["wxu"] = nc.dram_tensor("wxu", [128, 3, FOUR_H], BF16, kind="ExternalInput").ap()
    d["whu"] = nc.dram_tensor("whu", [128, 4, FOUR_H], BF16, kind="ExternalInput").ap()
    d["bu"] = nc.dram_tensor("bu", [128, FOUR_H], F32R, kind="ExternalInput").ap()
    d["wxd"] = nc.dram_tensor("wxd", [128, 4, FOUR_H], BF16, kind="ExternalInput").ap()
    d["whd"] = nc.dram_tensor("whd", [128, 4, FOUR_H], BF16, kind="ExternalInput").ap()
    d["bd"] = nc.dram_tensor("bd", [128, FOUR_H], F32R, kind="ExternalInput").ap()
    d["id2"] = nc.dram_tensor("id2", [128, 128], BF16, kind="ExternalInput").ap()
    d["init0"] = nc.dram_tensor("init0", [128, 1792], F32R, kind="ExternalInput").ap()
    d["init0b"] = nc.dram_tensor("init0b", [128, 256], BF16, kind="ExternalInput").ap()
    d["scatH"] = nc.dram_tensor("scatH", [128, T], I32, kind="ExternalInput").ap()
    d["hT"] = nc.dram_tensor("hT", [T, 128, 256], BF16, kind="Internal").ap()
    d["rmax"] = nc.dram_tensor("rmax", [B, H], F32R, kind="ExternalOutput").ap()

    hT_rows = d["hT"].rearrange("t p c -> (t p) c")

    def build_scan(tc, src, wh_name, wx_name, b_name, store_hT, rmax_out, lead):
        """One LSTM scan with its input projection fused into the schedule.

        src: "x" (xTu, 3 K-tiles, bf16) or "h" (hT, 4 K-tiles, f32r).
        proj_half(idx): idx=(m,nh) computes cols [nh*1024,(nh+1)*1024) of
        xz for timesteps 2m,2m+1 packed on ring partitions (t,b)."""
        KT = 3 if src == "x" else 4
        ring_bufs = lead // 4 + 3
        with (
            tc.tile_pool(name=f"w_{wh_name}", bufs=1) as whp,
            tc.tile_pool(name=f"st_{wh_name}", bufs=1) as statep,
            tc.tile_pool(name=f"src_{wh_name}", bufs=2) as srcp,
            tc.tile_pool(name=f"ring_{wh_name}", bufs=ring_bufs) as ringp,
            tc.tile_pool(name=f"g_{wh_name}", bufs=2) as gp,
            tc.tile_pool(name=f"z0_{wh_name}", bufs=1, space="PSUM") as z0p,
            tc.tile_pool(name=f"z1_{wh_name}", bufs=1, space="PSUM") as z1p,
            tc.tile_pool(name=f"z2_{wh_name}", bufs=1, space="PSUM") as z2p,
            tc.tile_pool(name=f"z3_{wh_name}", bufs=1, space="PSUM") as z3p,
            tc.tile_pool(name=f"tps_{wh_name}", bufs=1, space="PSUM") as tpsp,
            tc.tile_pool(name=f"pps_{wh_name}", bufs=2, space="PSUM") as projp,
        ):
            wh_sb = whp.tile([128, 4, FOUR_H], BF16, tag="wh")
            nc.sync.dma_start(wh_sb[:], d[wh_name])
            id2_sb = whp.tile([128, 2, 64], BF16, tag="id2")
            nc.sync.dma_start(id2_sb[:],
                              d["id2"].rearrange("p (j b) -> p j b", j=2))
            # id2_sb[:, 0, :] selects batch rows 0:63, [:, 1, :] rows 64:127
            # (used for inject and for transpose-by-identity-selection)
            if src == "x":
                wx_sb = whp.tile([128, 3, FOUR_H], BF16, tag="wx")
            else:
                wx_sb = whp.tile([128, 4, FOUR_H], BF16, tag="wx")
            nc.sync.dma_start(wx_sb[:], d[wx_name])
            b_sb = whp.tile([128, FOUR_H], F32R, tag="b")
            nc.sync.dma_start(b_sb[:], d[b_name])
            if store_hT:
                scatH_sb = whp.tile([128, T], I32, tag="scatH")
                nc.sync.dma_start(scatH_sb[:], d["scatH"])

            # hT in two half-tiles (k 0,1 | k 2,3) so next-step hWh k-tiles
            # unblock per-half as the tail finishes each copy
            hTa = statep.tile([128, 2, 64], BF16, tag="hTa")
            hTb = statep.tile([128, 2, 64], BF16, tag="hTb")
            hT_half = [hTa, hTb]
            # split-H layout: all gate/state tensors are [128, 256] with
            # partition p<64 = (batch p, h-cols 0:256) and p>=64 =
            # (batch p-64, h-cols 256:512). The two h-halves ride the two
            # 128x64 column tiles of the PE array concurrently, and every
            # ACT/DVE op runs 128 lanes wide at half the columns.
            c_sb = statep.tile([128, 256], F32R, tag="c")
            tg_sb = statep.tile([128, 256], F32R, tag="tg")
            # walrus rejects DVE memset on f32r - init state via DMA instead
            nc.sync.dma_start(
                hTa[:], d["init0b"][:, 0:128].rearrange("p (k b) -> p k b", k=2))
            nc.sync.dma_start(
                hTb[:], d["init0b"][:, 128:256].rearrange("p (k b) -> p k b", k=2))
            nc.sync.dma_start(c_sb[:], d["init0"][:, 256:512])
            if rmax_out is not None:
                rmax_sb = statep.tile([128, 256], F32R, tag="rmax")
                nc.sync.dma_start(rmax_sb[:], d["init0"][:, 1280:1536])

            src_tiles, zs_pairs = {}, {}

            def proj_quarter(idx):
                # one 512-col gate-bank j of the 2-timestep pair m; zp is a
                # single PSUM bank double-buffered so consecutive quarters'
                # matmuls never WAR-stall on the previous bias-add
                m, j = divmod(idx, 4)
                if j == 0:
                    if src == "x":
                        st_t = srcp.tile([128, 3, 2, 64], BF16, tag="srct")
                        nc.sync.dma_start(
                            st_t[:],
                            d["xTu"][2 * m:2 * m + 2].rearrange(
                                "t p k b -> p k t b"))
                    else:
                        st_t = srcp.tile([128, 4, 2, 64], BF16, tag="srct")
                        nc.sync.dma_start(
                            st_t[:],
                            d["hT"][2 * m:2 * m + 2].rearrange(
                                "t p (k b) -> p k t b", k=4))
                    src_tiles[m] = st_t
                    zs_t = ringp.tile([128, 4, 512], BF16, tag="zs")
                    zs_pairs[m] = zs_t
                st_t, zs = src_tiles[m], zs_pairs[m]
                zp = projp.tile([128, 512], F32, tag="zp")
                for k in range(KT):
                    nc.tensor.matmul(
                        zp[:],
                        st_t[:, k, :, :].rearrange("p t b -> p (t b)"),
                        wx_sb[:, k, bass.ts(j, 512)],
                        start=(k == 0), stop=(k == KT - 1))
                nc.vector.tensor_add(
                    zs[:, j, :], zp[:], b_sb[:, bass.ts(j, 512)])
                if j == 3:
                    del src_tiles[m]

            def alloc_inject(t):
                # xz lands in PSUM via identity matmul, off the h-dependency
                # chain; emitted one step ahead as PE filler during the tail.
                # Each bank is a [128, 256] tile: h-cols 0:256 land on PSUM
                # partitions 0:63 (column tile T0), h-cols 256:512 on 64:127
                # (column tile T1) - the two matmuls run concurrently.
                m, j2 = divmod(t, 2)
                zs = zs_pairs[m]
                z0 = z0p.tile([128, 256], F32, tag="z0")
                z1 = z1p.tile([128, 256], F32, tag="z1")
                z2 = z2p.tile([128, 256], F32, tag="z2")
                z3 = z3p.tile([128, 256], F32, tag="z3")
                zb = [z0, z1, z2, z3]  # per-bank tiles: acts start per-bank
                for n in range(4):
                    nc.tensor.matmul(zb[n][0:64, :], id2_sb[:, j2, :],
                                     zs[:, n, 0:256], start=True, stop=False)
                    nc.tensor.matmul(zb[n][64:128, :], id2_sb[:, j2, :],
                                     zs[:, n, 256:512], start=True, stop=False)
                if j2 == 1:
                    del zs_pairs[m]
                return zb

            for idx in range(lead):
                proj_quarter(idx)

            zb = alloc_inject(0)
            for t in range(T):
                for n in range(4):
                    for k in range(4):
                        half, kk = hT_half[k // 2], k % 2
                        nc.tensor.matmul(
                            zb[n][0:64, :], half[:, kk, :],
                            wh_sb[:, k, n * 512:n * 512 + 256],
                            start=False, stop=(k == 3))
                        nc.tensor.matmul(
                            zb[n][64:128, :], half[:, kk, :],
                            wh_sb[:, k, n * 512 + 256:n * 512 + 512],
                            start=False, stop=(k == 3))
                for i in range(2):
                    if 2 * t + i + lead < 2 * T:
                        proj_quarter(2 * t + i + lead)
                # gate chain, bank-split: sig_f -> f*c while later banks
                # still stream on the PE; all ops 128 lanes x 256 cols
                sf = gp.tile([128, 256], F32R, tag="sf")
                nc.scalar.activation(sf[:], zb[0][:], Sig)
                si = gp.tile([128, 256], F32R, tag="si")
                nc.scalar.activation(si[:], zb[1][:], Sig)
                m1 = gp.tile([128, 256], F32R, tag="m1")
                nc.vector.tensor_mul(m1[:], sf[:], c_sb[:])  # f*c
                nc.scalar.activation(tg_sb[:], zb[2][:], Tanh)
                go = gp.tile([128, 256], F32R, tag="go")
                nc.scalar.activation(go[:], zb[3][:], Sig)
                zb_next = alloc_inject(t + 1) if t + 1 < T else None
                m2 = gp.tile([128, 256], F32R, tag="m2")
                nc.vector.tensor_mul(m2[:], si[:], tg_sb[:])  # i*tg
                nc.vector.tensor_add(c_sb[:], m1[:], m2[:])
                tc_t = gp.tile([128, 256], F32R, tag="tc")
                nc.scalar.activation(tc_t[:], c_sb[:], Tanh)
                h_t = gp.tile([128, 256], BF16, tag="h")
                nc.vector.tensor_mul(h_t[:], go[:], tc_t[:])
                if store_hT or t + 1 < T:
                    # transpose-by-identity-selection: out = h_slice^T @
                    # id-block extracts one batch-half transposed, as plain
                    # (0,0) matmuls (row-tiled transposes crash walrus).
                    # k0/k2 and k1/k3 share the stationary h_t slice.
                    pTa = tpsp.tile([128, 2, 64], F32, tag="pT0")
                    pTb = tpsp.tile([128, 2, 64], F32, tag="pT1")
                    for k in range(2):
                        nc.tensor.matmul(pTa[:, k, :], h_t[:, bass.ts(k, 128)],
                                         id2_sb[:, 0, :], start=True, stop=True)
                        nc.tensor.matmul(pTb[:, k, :], h_t[:, bass.ts(k, 128)],
                                         id2_sb[:, 1, :], start=True, stop=True)
                    nc.scalar.activation(hTa[:], pTa[:], Copy)
                    nc.scalar.activation(hTb[:], pTb[:], Copy)
                if store_hT:
                    for q in range(2):
                        nc.gpsimd.indirect_dma_start(
                            out=hT_rows,
                            out_offset=bass.IndirectOffsetOnAxis(
                                ap=scatH_sb[:, t:t + 1], axis=0),
                            in_=hT_half[q][:].rearrange("p k b -> p (k b)"),
                            in_offset=None,
                            element_offset=128 * q)
                if rmax_out is not None:
                    nc.vector.tensor_max(rmax_sb[:], rmax_sb[:], h_t[:])
                zb = zb_next
            if rmax_out is not None:
                # rmax_sb[p<64] = (batch p, h 0:256); [p>=64] = h 256:512
                nc.sync.dma_start(rmax_out[:, 0:256], rmax_sb[0:64, :])
                nc.sync.dma_start(rmax_out[:, 256:512], rmax_sb[64:128, :])

    with tile.TileContext(nc) as tc:
        for _ in range(rep):
            build_scan(tc, "x", "whu", "wxu", "bu", store_hT=True,
                       rmax_out=None, lead=16)
            build_scan(tc, "h", "whd", "wxd", "bd", store_hT=False,
                       rmax_out=d["rmax"], lead=8)
    nc.compile()
    return nc


class Runner:
    """Trace/lower/compile the program once; reuse the executable."""

    def __init__(self, nc):
        import jax
        from jax.sharding import Mesh, PartitionSpec, NamedSharding
        from jax.experimental.shard_map import shard_map
        from concourse import mybir
        from concourse.bass2jax import _bass_exec_p, install_neuronx_cc_hook

        install_neuronx_cc_hook()
        self.jax = jax
        partition_name = (nc.partition_id_tensor.name
                          if nc.partition_id_tensor else None)
        in_names, out_names, out_avals, zero_shapes = [], [], [], []
        for alloc in nc.m.functions[0].allocations:
            if not isinstance(alloc, mybir.MemoryLocationSet):
                continue
            name = alloc.memorylocations[0].name
            if alloc.kind == "ExternalInput":
                if name != partition_name:
                    in_names.append(name)
            elif alloc.kind == "ExternalOutput":
                out_names.append(name)
                out_avals.append(jax.core.ShapedArray(
                    tuple(alloc.tensor_shape), mybir.dt.np(alloc.dtype)))
        self.in_names, self.out_names, self.out_avals = \
            in_names, out_names, out_avals
        n_params = len(in_names)
        all_in = list(in_names) + list(out_names)
        if partition_name is not None:
            all_in.append(partition_name)

        def _body(*args):
            operands = list(args)
            if partition_name is not None:
                from concourse.bass2jax import partition_id_tensor
                operands.append(partition_id_tensor())
            return tuple(_bass_exec_p.bind(
                *operands, out_avals=tuple(out_avals), in_names=tuple(all_in),
                out_names=tuple(out_names), lowering_input_output_aliases=(),
                sim_require_finite=True, sim_require_nnan=True, nc=nc))

        devices = jax.devices()[:N_CORES]
        assert len(devices) == N_CORES
        self.mesh = Mesh(np.asarray(devices), ("core",))
        self.spec = NamedSharding(self.mesh, PartitionSpec("core"))
        self.sharded = jax.jit(
            shard_map(_body, mesh=self.mesh,
                      in_specs=(PartitionSpec("core"),) * (n_params + len(out_names)),
                      out_specs=(PartitionSpec("core"),) * len(out_names),
                      check_rep=False),
            keep_unused=True)

    def put_inputs(self, in_maps):
        dev_in = []
        for nm in self.in_names:
            cat = np.concatenate(
                [np.asarray(in_maps[c][nm]) for c in range(N_CORES)], axis=0)
            dev_in.append(self.jax.device_put(cat, self.spec))
        dev_zero = [
            self.jax.device_put(
                np.zeros((N_CORES * a.shape[0], *a.shape[1:]), a.dtype),
                self.spec)
            for a in self.out_avals]
        self.jax.block_until_ready(dev_in)
        return dev_in, dev_zero

    def call(self, dev_in, dev_zero):
        out = self.sharded(*dev_in, *dev_zero)
        self.jax.block_until_ready(out)
        return out

    def fetch(self, out):
        return [
            {nm: np.asarray(out[i]).reshape(N_CORES, *self.out_avals[i].shape)[c]
             for i, nm in enumerate(self.out_names)}
            for c in range(N_CORES)]


def _get_runner(rep=1):
    key = f"runner{rep}"
    if key not in _CACHE:
        _CACHE[key] = Runner(_build_program(rep))
    return _CACHE[key]


def _prep_shared(x):
    """x [B,T,E] -> xT [T,128,3,64] bf16, xT[t,p,k,b] = x[b,t,k*128+p]."""
    try:
        import ml_dtypes
        bf16 = ml_dtypes.bfloat16
    except ImportError:
        import jax.numpy as jnp
        bf16 = jnp.bfloat16
    xpad = np.zeros((B, T, 384), np.float32)
    xpad[:, :, :E] = x
    xT = xpad.transpose(1, 2, 0).reshape(T, 3, 128, B).transpose(0, 2, 1, 3)
    return np.ascontiguousarray(xT.astype(bf16))


_GATE_PERM = np.r_[H:2 * H, 0:H, 2 * H:3 * H, 3 * H:4 * H]  # [i f g o]->[f i g o]

_INIT0 = np.zeros((128, 1792), np.float32)
_INIT0[:, 1280:1536] = -1e30


def _prep_core(xT, wx_u, wh_u, b_u, wx_d, wh_d, b_d, reverse):
    wx_u = np.asarray(wx_u, np.float32)[:, _GATE_PERM]
    wh_u = np.asarray(wh_u, np.float32)[:, _GATE_PERM]
    b_u = np.asarray(b_u, np.float32)[_GATE_PERM]
    wx_d = np.asarray(wx_d, np.float32)[:, _GATE_PERM]
    wh_d = np.asarray(wh_d, np.float32)[:, _GATE_PERM]
    b_d = np.asarray(b_d, np.float32)[_GATE_PERM]
    wxu_pad = np.zeros((384, FOUR_H), np.float32)
    wxu_pad[:E] = wx_u
    p = np.arange(128)
    t = np.arange(T)
    t_dst = (T - 1 - t) if reverse else t
    scatH = (t_dst[None, :] * 128 + p[:, None]).astype(np.int32)
    wxu = np.ascontiguousarray(wxu_pad.reshape(3, 128, FOUR_H).transpose(1, 0, 2))
    return {
        "xTu": xT,
        "wxu": wxu.astype(xT.dtype),
        "whu": np.ascontiguousarray(
            np.asarray(wh_u, np.float32).reshape(4, 128, FOUR_H).transpose(1, 0, 2)).astype(xT.dtype),
        "bu": np.ascontiguousarray(
            np.broadcast_to(np.asarray(b_u, np.float32), (128, FOUR_H))),
        "wxd": np.ascontiguousarray(
            np.asarray(wx_d, np.float32).reshape(4, 128, FOUR_H).transpose(1, 0, 2)).astype(xT.dtype),
        "whd": np.ascontiguousarray(
            np.asarray(wh_d, np.float32).reshape(4, 128, FOUR_H).transpose(1, 0, 2)).astype(xT.dtype),
        "bd": np.ascontiguousarray(
            np.broadcast_to(np.asarray(b_d, np.float32), (128, FOUR_H))),
        "id2": np.eye(128, dtype=np.float32).astype(xT.dtype),
        "init0": _INIT0,
        "init0b": np.zeros((128, 256), np.float32).astype(xT.dtype),
        "scatH": scatH,
    }


def build_in_maps(x, uni_Wx, uni_Wh, uni_b, fwd_Wx, fwd_Wh, fwd_b,
                  bwd_Wx, bwd_Wh, bwd_b):
    xT = _prep_shared(np.asarray(x, np.float32))
    in_maps = []
    for c in range(N_CORES):
        br = c % 3
        if c < 3:
            wx_d, wh_d, b_d, rev = fwd_Wx[br], fwd_Wh[br], fwd_b[br], False
        else:
            wx_d, wh_d, b_d, rev = bwd_Wx[br], bwd_Wh[br], bwd_b[br], True
        in_maps.append(_prep_core(xT, np.asarray(uni_Wx[br], np.float32),
                                  uni_Wh[br], uni_b[br], wx_d, wh_d, b_d, rev))
    return in_maps


def kernel(x, uni_Wx, uni_Wh, uni_b, fwd_Wx, fwd_Wh, fwd_b,
           bwd_Wx, bwd_Wh, bwd_b):
    in_maps = build_in_maps(x, uni_Wx, uni_Wh, uni_b, fwd_Wx, fwd_Wh, fwd_b,
                            bwd_Wx, bwd_Wh, bwd_b)
    r = _get_runner(rep=1)
    dev_in, dev_zero = r.put_inputs(in_maps)
    res = r.fetch(r.call(dev_in, dev_zero))
    out = np.empty((B, NB * 2 * H), np.float32)
    for c in range(N_CORES):
        br = c % 3
        off = br * 2 * H + (0 if c < 3 else H)
        out[:, off:off + H] = res[c]["rmax"]
    return out
